# revision 7
# baseline (speedup 1.0000x reference)
"""Cross-attention layer kernel for Trainium2 (Bass/Tile), 8-core data-parallel.

Computes, per batch element b (one NeuronCore each):
    Q = Wq @ Xq + bq            (64, HW)     1x1 conv == channel matmul
    K = Wk @ Xk + bk            (64, HW)
    S^T = K^T Q                 (HW, HW)     keys on partitions
    P^T = exp(S^T) / l          l[i] = sum_p exp(S^T[p,i])
    out = V P^T, V = Xk         (C, HW)

Dims: B=8, C=512, H=W=64 -> HW=4096, D=64.

v4: everything on the PE is bf16 (host ships bf16 Xq/Xk/W; validated
8.5e-3 rel err vs the 2e-2 gate) -- no dtype mode switches, half the DMA
and SBUF stream bytes. S^T is computed transposed so exp(S^T) feeds PV
directly (no P^T transposes). The softmax denominator is accumulated on
the PE itself: a ones[128,1] stationary contracts each exp chunk into a
[1,512] PSUM tile (2 extra 512-col matmuls per exp pair -- far cheaper
than DVE/GPSIMD reductions, and the result lands ~1us after the last
exp, so 1/l never stalls the PV evacuation). reciprocal_approx_fast +
GPSIMD partition_broadcast produce the [128,512] 1/l operand; DVE only
does bias adds, V^T evac copies, and the PV evacuation multiplies.
S^T(qs)+exp pairs, l-matmuls, and PV(qs-1) interleave in emission order
so ACT exp latency hides under PV work; xq projections fill qs==0.
"""

import numpy as np

try:
    import concourse.bass as bass
except ImportError:  # pragma: no cover - path setup for bare containers
    import sys

    sys.path.insert(0, "/opt/trn_rl_repo")
    import concourse.bass as bass

import ml_dtypes
import concourse.mybir as mybir
import concourse.tile as tile
from concourse import bacc, bass_isa
from concourse.bass_utils import run_bass_kernel_spmd
from concourse.masks import make_identity

F32 = mybir.dt.float32
BF16 = mybir.dt.bfloat16
AF = mybir.ActivationFunctionType
AX = mybir.AxisListType

B = 8
C = 512
HW = 4096
D = 64
N_CORES = 8

XP_BUFS = 4
S_PS_BUFS = 2


def build_nc(c=C, hw=HW, d=D):
    """Build the single-core Bass program (SPMD across cores via inputs)."""
    P = 128
    NKC = c // P          # channel chunks (contraction for projections)
    NSLAB = hw // 512     # 512-wide column slabs (proj n-tiles / q-supers)
    NPC = hw // P         # 128-wide key chunks (S^T partition blocks)
    NPAIR = NPC // 2      # exp pairs per q-super

    nc = bacc.Bacc("TRN2", target_bir_lowering=False)

    xq = nc.dram_tensor("xq", [c, hw], BF16, kind="ExternalInput")
    xk = nc.dram_tensor("xk", [c, hw], BF16, kind="ExternalInput")
    wqt = nc.dram_tensor("wqt", [c, d], BF16, kind="ExternalInput")
    wkt = nc.dram_tensor("wkt", [c, d], BF16, kind="ExternalInput")
    bq = nc.dram_tensor("bq", [d, 1], F32, kind="ExternalInput")
    bk = nc.dram_tensor("bk", [d, 1], F32, kind="ExternalInput")
    out = nc.dram_tensor("out", [c, hw], F32, kind="ExternalOutput")

    with tile.TileContext(nc) as tc:
        with (
            tc.tile_pool(name="const", bufs=1) as const,
            tc.tile_pool(name="persist", bufs=1) as persist,
            tc.tile_pool(name="psA", bufs=2, space="PSUM") as psA,
        ):
            # ---- constants ----
            ident = const.tile([P, P], BF16, name="ident")
            make_identity(nc, ident)
            ones_sb = const.tile([P, 1], BF16, name="ones_sb")
            nc.vector.memset(ones_sb, 1.0)
            wq_sb = const.tile([P, NKC, d], BF16, name="wq_sb")
            nc.sync.dma_start(
                out=wq_sb, in_=wqt[:, :].rearrange("(n p) d -> p n d", p=P)
            )
            wk_sb = const.tile([P, NKC, d], BF16, name="wk_sb")
            nc.sync.dma_start(
                out=wk_sb, in_=wkt[:, :].rearrange("(n p) d -> p n d", p=P)
            )
            bq_sb = const.tile([d, 1], F32, name="bq_sb")
            nc.sync.dma_start(out=bq_sb, in_=bq[:, :])
            bk_sb = const.tile([d, 1], F32, name="bk_sb")
            nc.sync.dma_start(out=bk_sb, in_=bk[:, :])

            # persistent activations: Q/K rows 0:64 live, 64:128 duplicate
            q_sb = persist.tile([P, hw], BF16, name="q_sb")
            k_sb = persist.tile([P, hw], BF16, name="k_sb")
            vt_sb = persist.tile([P, NPC, c], BF16, name="vt_sb")  # V^T = Xk^T

            # ============ phase 1: load + projections + V^T ============
            with (
                tc.tile_pool(name="xp", bufs=XP_BUFS) as xp,
                tc.tile_pool(name="psT", bufs=2, space="PSUM") as psT,
            ):
                def proj_slab(x_dram, w_sb, b_sb, dst, n):
                    sl = slice(n * 512, (n + 1) * 512)
                    xt = xp.tile([P, NKC, 512], BF16, name="xt", tag="xt")
                    xr = x_dram[:, :].rearrange("(a p) q -> p a q", p=P)[:, :, sl]
                    for kc in range(NKC):
                        nc.sync.dma_start(
                            out=xt[:, kc : kc + 1, :], in_=xr[:, kc : kc + 1, :]
                        )
                    ps = psA.tile([d, 512], F32, name="proj_ps", tag="psA")
                    for kc in range(NKC):
                        nc.tensor.matmul(
                            ps,
                            w_sb[:, kc, :],
                            xt[:, kc, :],
                            start=(kc == 0),
                            stop=(kc == NKC - 1),
                        )
                    # evacuate + bias (DVE), duplicate rows 64:128 (DMA)
                    nc.vector.tensor_scalar_add(dst[0:d, sl], ps, b_sb)
                    nc.sync.dma_start(out=dst[d : 2 * d, sl], in_=dst[0:d, sl])
                    return xt

                for n in range(NSLAB):
                    xt = proj_slab(xk, wk_sb, bk_sb, k_sb, n)
                    for j in range(512 // P):
                        pc = n * (512 // P) + j
                        tp = psT.tile([P, c], BF16, name="vt_ps", tag="psT")
                        for kc in range(NKC):
                            nc.tensor.transpose(
                                tp[:, kc * P : (kc + 1) * P],
                                xt[:, kc, j * P : (j + 1) * P],
                                ident,
                            )
                        nc.vector.tensor_copy(vt_sb[:, pc, :], tp)
                proj_slab(xq, wq_sb, bq_sb, q_sb, 0)

            # ============ phase 2: attention (pipelined q-supers) ======
            with (
                tc.tile_pool(name="estp", bufs=2) as estp,
                tc.tile_pool(name="lp", bufs=2) as lp,
                tc.tile_pool(name="outp", bufs=4) as outp,
                tc.tile_pool(name="psS", bufs=1, space="PSUM") as psS,
                tc.tile_pool(name="psV", bufs=2, space="PSUM") as psV,
                tc.tile_pool(name="psL", bufs=2, space="PSUM") as psL,
                tc.tile_pool(name="xp2", bufs=2) as xp2,
            ):
                def emit_st_exp_pair(qs, est, k):
                    """S^T matmuls for key chunks (2k, 2k+1) + one 1024-wide exp."""
                    qsl = slice(qs * 512, (qs + 1) * 512)
                    sp = psS.tile([P, 2, 512], F32, name="s_ps", tag="psS")
                    for half in range(2):
                        pc = 2 * k + half
                        h = (pc % 2) * d
                        nc.tensor.matmul(
                            sp[:, half, :],
                            k_sb[h : h + d, pc * P : (pc + 1) * P],
                            q_sb[h : h + d, qsl],
                            start=True,
                            stop=True,
                        )
                    nc.scalar.activation(est[:, 2 * k : 2 * k + 2, :], sp, AF.Exp)

                def emit_l_pair(est, psl, k):
                    """PE ones-contraction of exp pair k into the [1,512] l psum."""
                    for half in range(2):
                        pc = 2 * k + half
                        nc.tensor.matmul(
                            psl,
                            ones_sb,
                            est[:, pc, :],
                            start=(pc == 0),
                            stop=(pc == NPC - 1),
                        )

                def emit_l_tail(psl):
                    linv1 = lp.tile([1, 512], F32, name="linv1", tag="linv1")
                    nc.vector.reciprocal_approx_fast(linv1, psl)
                    linv = lp.tile([P, 512], F32, name="linv", tag="linv")
                    nc.gpsimd.partition_broadcast(linv, linv1, channels=P)
                    return linv

                def emit_pv_slot(qs, est, linv, slot, st):
                    """8 PV matmuls (flattened over ct,pc); evac when a ct ends."""
                    qsl = slice(qs * 512, (qs + 1) * 512)
                    for i in range(slot * 8, slot * 8 + 8):
                        ct, pc = i // NPC, i % NPC
                        if pc == 0:
                            st["ops"] = psV.tile([P, 512], F32, name="pv_ps", tag="psV")
                        nc.tensor.matmul(
                            st["ops"],
                            vt_sb[:, pc, ct * P : (ct + 1) * P],
                            est[:, pc, :],
                            start=(pc == 0),
                            stop=(pc == NPC - 1),
                        )
                        if pc == NPC - 1:
                            ot = outp.tile([P, 512], F32, name="ot", tag="ot")
                            nc.vector.tensor_mul(ot, st["ops"], linv)
                            nc.sync.dma_start(
                                out=out[ct * P : (ct + 1) * P, qsl], in_=ot
                            )

                def proj_q_slab(n):
                    sl = slice(n * 512, (n + 1) * 512)
                    xt = xp2.tile([P, NKC, 512], BF16, name="xt2", tag="xt2")
                    xr = xq[:, :].rearrange("(a p) q -> p a q", p=P)[:, :, sl]
                    for kc in range(NKC):
                        nc.sync.dma_start(
                            out=xt[:, kc : kc + 1, :], in_=xr[:, kc : kc + 1, :]
                        )
                    ps = psA.tile([d, 512], F32, name="proj_ps2", tag="psA")
                    for kc in range(NKC):
                        nc.tensor.matmul(
                            ps,
                            wq_sb[:, kc, :],
                            xt[:, kc, :],
                            start=(kc == 0),
                            stop=(kc == NKC - 1),
                        )
                    nc.vector.tensor_scalar_add(q_sb[0:d, sl], ps, bq_sb)
                    nc.sync.dma_start(out=q_sb[d : 2 * d, sl], in_=q_sb[0:d, sl])

                pv_state = {}
                prev = None  # (est, linv, qs)
                for qs in range(NSLAB):
                    est = estp.tile([P, NPC, 512], BF16, name="est", tag="est")
                    psl = psL.tile([1, 512], F32, name="l_ps", tag="psL")
                    for k in range(NPAIR):
                        emit_st_exp_pair(qs, est, k)
                        if k > 0:
                            emit_l_pair(est, psl, k - 1)
                        if prev is not None:
                            emit_pv_slot(prev[2], prev[0], prev[1], k, pv_state)
                        elif qs == 0 and k % 2 == 0 and 1 + k // 2 < NSLAB:
                            proj_q_slab(1 + k // 2)  # xq slabs 1..7 fill qs==0
                    emit_l_pair(est, psl, NPAIR - 1)
                    linv = emit_l_tail(psl)
                    prev = (est, linv, qs)
                # drain: PV for the last q-super
                for slot in range(NPAIR):
                    emit_pv_slot(prev[2], prev[0], prev[1], slot, pv_state)

    nc.compile()
    return nc


_NC_CACHE = {}


def _get_nc():
    key = (C, HW, D)
    if key not in _NC_CACHE:
        _NC_CACHE[key] = build_nc()
    return _NC_CACHE[key]


def make_in_maps(query_features, key_features, Wq, bq, Wk, bk):
    bf16 = ml_dtypes.bfloat16
    query_features = np.asarray(query_features, dtype=np.float32)
    key_features = np.asarray(key_features, dtype=np.float32)
    wqt = np.ascontiguousarray(np.asarray(Wq, dtype=np.float32).T.astype(bf16))
    wkt = np.ascontiguousarray(np.asarray(Wk, dtype=np.float32).T.astype(bf16))
    bq_ = np.ascontiguousarray(np.asarray(bq, dtype=np.float32).reshape(D, 1))
    bk_ = np.ascontiguousarray(np.asarray(bk, dtype=np.float32).reshape(D, 1))
    in_maps = []
    for b in range(B):
        in_maps.append(
            {
                "xq": np.ascontiguousarray(
                    query_features[b].reshape(C, HW).astype(bf16)
                ),
                "xk": np.ascontiguousarray(
                    key_features[b].reshape(C, HW).astype(bf16)
                ),
                "wqt": wqt,
                "wkt": wkt,
                "bq": bq_,
                "bk": bk_,
            }
        )
    return in_maps


def kernel(query_features, key_features, Wq, bq, Wk, bk, vis_CA=0, **_unused):
    nc = _get_nc()
    in_maps = make_in_maps(query_features, key_features, Wq, bq, Wk, bk)
    res = run_bass_kernel_spmd(nc, in_maps, core_ids=list(range(N_CORES)))
    h = int(np.sqrt(HW))
    outs = [r["out"].reshape(C, h, h) for r in res.results]
    return np.stack(outs).astype(np.float32)
